# revision 13
# baseline (speedup 1.0000x reference)
"""Correlation-layer (cost volume) kernel for 8 Trainium2 NeuronCores.

Problem: out[n, 0, h, w, dy*41+dx] = sum_c fm1[n,c,h,w] * fm2p[n,c,h+dy,w+dx]
with fm2p = fm2 zero-padded by 20 on both spatial axes, dy,dx in [0,41).

Sharding: core k handles batch n = k//2 and h-slab [64*(k%2), 64*(k%2)+64).
No cross-core communication: each core's fm2 slab (with a 20-row halo) is
prepared on the host.

Device algorithm (per core, fp16 in / fp32 PSUM / fp16 out), v2:
  - Stationary = a 16x8 (h,w)-BLOCK of fm1 (K=64 channels, M=128 = 16*8
    pixels).  One stationary load then serves all 41*41 displacements for
    128 output pixels: the moving stream is fm2 rows [h0, h0+56) x padded
    cols [w0, w0+48), i.e. (16+40)*(8+40) = 2688 columns per load instead
    of the 8528 the per-row band formulation needs.  64 loads total.
  - Moving is streamed in 7 chunks of 8 rows x 48 cols = 384 fp32 columns,
    each into its own PSUM bank; PSUM[p = h_in*8+w_in, r_rel*48 + w_rel] =
    <fm1[:,h,w], fm2[:,h0+r_rel,w0+w_rel]>.
  - PSUM is evacuated (fp32->fp16) into an SBUF band tile S[128, 56, 48]
    split across DVE / ACT / Pool so no single engine bottlenecks.
  - Output DMA per load either writes the full band (TRIM=False) or uses a
    custom 3-dim addr64 access pattern whose outer stride advances 8
    partitions AND one 48-col band row at once (TRIM=True), storing only
    the 41 rows [h_in, h_in+41) each partition actually needs.
  - The w-direction shear (dx = w' - w_in) cannot be expressed in <=3 DMA
    dims; the host extracts the 41 diagonal columns with a zero-copy
    as_strided view during the fp32 upcast.
"""

import os
import sys

import numpy as np

for _p in ("/opt/trn_rl_repo",):
    if os.path.isdir(_p) and _p not in sys.path:
        sys.path.append(_p)

# ---- problem constants (hardcoded per contest rules) ----
B, C, H, W = 4, 64, 128, 128
MD = 20                  # max displacement
D = 2 * MD + 1           # 41 displacements per axis
PW = W + 2 * MD          # 168 padded width
HS = H // 2              # 64-row h-slab per core
RS = HS + 2 * MD         # 104 fm2 slab rows (with halo)
NCORES = 8

NH, NW = 16, 8           # fm1 pixel block per stationary load (NH*NW = 128)
NHB, NWB = HS // NH, W // NW      # 4 x 16 = 64 loads
BR, BC = NH + 2 * MD, NW + 2 * MD  # 56 x 48 band per load
CH = 8                   # moving rows per matmul chunk
NCH = BR // CH           # 7 chunks/load, 8*48 = 384 fp32 <= 512 (1 PSUM bank)

TRIM = True              # row-trimmed output DMA via custom addr64 AP

_CACHE = {}


def _patch_ldw_opt():
    """Flip walrus's --enable-ldw-opt to true for our compiles: all 7 matmuls
    of a load share one stationary, and without the opt each one pays a
    ~146ns LDWEIGHTS that serializes with streaming (~40% of PE time)."""
    from concourse import bass_utils

    if getattr(bass_utils, "_ldw_patched", False):
        return
    # NOTE: --enable-ldw-opt=true breaks walrus codegen (visitInstLdweights
    # error), so the redundant per-matmul LDWEIGHTS stays; it overlaps the
    # previous matmul's streaming via the PE shadow weight buffer.
    bass_utils._ldw_patched = True


def _build_program(io_dtype_name="float16", trim=TRIM):
    from concourse import bacc
    import concourse.mybir as mybir
    import concourse.tile as tile

    _patch_ldw_opt()

    dt_io = getattr(mybir.dt, io_dtype_name)

    nc = bacc.Bacc("TRN2", target_bir_lowering=False, debug=False)
    # fm1 host-bl blocked: [c, hb, wb, h_in*NW + w_in] so a stationary load is
    # one contiguous 128-wide slice (BIR: stationary AP = single free dim).
    fm1_d = nc.dram_tensor(
        "fm1s", [C, NHB, NWB, NH * NW], dt_io, kind="ExternalInput"
    ).ap()
    fm2_d = nc.dram_tensor("fm2s", [C, RS, PW], dt_io, kind="ExternalInput").ap()
    # trim: partitions 0-63 (h_in 0-7) only need band rows [0,48); partitions
    # 64-127 (h_in 8-15) only rows [8,56).  Two half-DMAs per load store
    # 48x48 per pixel instead of 56x48.
    TR = BR - CH         # 48 trimmed rows per half
    if trim:
        out_shape = [NHB * NWB, 2, 64, TR * BC]
    else:
        out_shape = [NHB * NWB, NH * NW, BR * BC]
    out_d = nc.dram_tensor("outs", out_shape, dt_io, kind="ExternalOutput").ap()

    with tile.TileContext(nc) as tc:
        with (
            tc.tile_pool(name="const", bufs=1) as cpool,
            tc.tile_pool(name="srow", bufs=3) as spool,
            tc.tile_pool(name="psum", bufs=1, space="PSUM") as ppool,
        ):
            fm1_sb = cpool.tile([C, NHB, NWB, NH * NW], dt_io)
            fm2_sb = cpool.tile([C, RS, PW], dt_io)
            nc.sync.dma_start(fm1_sb[:], fm1_d[:])
            nc.sync.dma_start(fm2_sb[:], fm2_d[:])

            # PSUM: two tile groups per load (3 + 4 chunk slots, 512 fp32
            # each so every matmul lands in one bank).  Evacuation is two
            # big strided copies (DVE group 0, ACT group 1) instead of 7
            # small ones -- per-instruction overhead was ~45% of evac time.
            GSZ = (3, 4)
            for hb in range(NHB):
                for wb in range(NWB):
                    li = hb * NWB + wb
                    S = spool.tile([128, BR, BC], dt_io, tag="S")
                    j = 0
                    for gi, gn in enumerate(GSZ):
                        ps = ppool.tile(
                            [128, gn, 512], mybir.dt.float32,
                            name=f"ps{gi}", tag=f"ps{gi}",
                        )
                        for sl in range(gn):
                            nc.tensor.matmul(
                                ps[:, sl, 0 : CH * BC],
                                fm1_sb[:, hb, wb, :],
                                fm2_sb[:, NH * hb + CH * j
                                       : NH * hb + CH * (j + 1),
                                       NW * wb : NW * wb + BC],
                                start=True,
                                stop=True,
                            )
                            j += 1
                        dst = S[:, CH * (j - gn) : CH * j, :].rearrange(
                            "p (g r) c -> p g (r c)", g=gn
                        )
                        copy = (
                            nc.vector.tensor_copy if gi == 0
                            else nc.scalar.copy
                        )
                        copy(dst, ps[:, :, 0 : CH * BC])
                    if trim:
                        nc.sync.dma_start(out_d[li, 0], S[0:64, 0:TR, :])
                        nc.gpsimd.dma_start(out_d[li, 1], S[64:128, CH:BR, :])
                    else:
                        nc.sync.dma_start(out_d[li], S[:])

    nc.compile()
    return nc


def _get_compiled(io_dtype_name="float16", trim=TRIM):
    key = ("prog", io_dtype_name, trim)
    if key not in _CACHE:
        _CACHE[key] = _build_program(io_dtype_name, trim)
    return _CACHE[key]


def shard_inputs(fm1, fm2, np_dtype=np.float16):
    """Full (4,64,128,128) inputs -> 8 per-core input dicts."""
    fm1 = np.asarray(fm1, dtype=np.float32)
    fm2 = np.asarray(fm2, dtype=np.float32)
    in_maps = []
    for k in range(NCORES):
        n, hbase = k // 2, (k % 2) * HS
        slab = fm1[n, :, hbase : hbase + HS].astype(np_dtype)  # (C, 64, 128)
        fm1s = np.ascontiguousarray(
            slab.reshape(C, NHB, NH, NWB, NW)
            .transpose(0, 1, 3, 2, 4)
            .reshape(C, NHB, NWB, NH * NW)
        )
        p = np.zeros((C, H + 2 * MD, PW), dtype=np_dtype)
        p[:, MD : MD + H, MD : MD + W] = fm2[n].astype(np_dtype)
        fm2s = np.ascontiguousarray(p[:, hbase : hbase + RS])  # (C, 104, 168)
        in_maps.append({"fm1s": fm1s, "fm2s": fm2s})
    return in_maps


def unshard_outputs(results, trim=TRIM):
    """8 per-core band outputs -> full (4,1,128,128,1681) fp32."""
    out = np.empty((B, 1, H, W, D * D), dtype=np.float32)
    for k in range(NCORES):
        n, hbase = k // 2, (k % 2) * HS
        g = np.asarray(results[k]["outs"])
        if trim:
            TR = BR - CH
            # [hb, wb, half, h_in', w_in, r - 8*half, w']
            a = g.reshape(NHB, NWB, 2, CH, NW, TR, BC)
            st = a.strides
            band = np.lib.stride_tricks.as_strided(
                a,
                shape=(NHB, NWB, 2, CH, NW, D, D),
                strides=(st[0], st[1], st[2], st[3] + st[5], st[4] + st[6],
                         st[5], st[6]),
            )
            out[n, 0, hbase : hbase + HS] = (
                band.transpose(0, 2, 3, 1, 4, 5, 6)
                .astype(np.float32)
                .reshape(HS, W, D * D)
            )
            continue
        else:
            a = g.reshape(NHB, NWB, NH, NW, BR, BC)
            st = a.strides
            band = np.lib.stride_tricks.as_strided(
                a,
                shape=(NHB, NWB, NH, NW, D, D),
                strides=(st[0], st[1], st[2] + st[4], st[3] + st[5],
                         st[4], st[5]),
            )
        out[n, 0, hbase : hbase + HS] = (
            band.transpose(0, 2, 1, 3, 4, 5)
            .astype(np.float32)
            .reshape(HS, W, D * D)
        )
    return out


def run_on_hw(in_maps, io_dtype_name="float16", trace=False, **kw):
    from concourse import bass_utils

    nc = _get_compiled(io_dtype_name)
    res = bass_utils.run_bass_kernel_spmd(
        nc, in_maps, list(range(NCORES)), trace=trace, **kw
    )
    return res


def kernel(feature_map_1, feature_map_2):
    in_maps = shard_inputs(feature_map_1, feature_map_2)
    res = run_on_hw(in_maps)
    return unshard_outputs(res.results)


if __name__ == "__main__":
    inputs = {
        "feature_map_1": np.random.randn(B, C, H, W).astype(np.float32),
        "feature_map_2": np.random.randn(B, C, H, W).astype(np.float32),
    }
    out = kernel(**inputs)
    print("kernel output", out.shape, out.dtype)


# revision 15
# speedup vs baseline: 1.2647x; 1.2647x over previous
"""Correlation-layer (cost volume) kernel for 8 Trainium2 NeuronCores.

Problem: out[n, 0, h, w, dy*41+dx] = sum_c fm1[n,c,h,w] * fm2p[n,c,h+dy,w+dx]
with fm2p = fm2 zero-padded by 20 on both spatial axes, dy,dx in [0,41).

Sharding: core k handles batch n = k//2 and h-slab [64*(k%2), 64*(k%2)+64).
No cross-core communication: each core's fm2 slab (with a 20-row halo) is
prepared on the host.

Device algorithm (per core, fp16 in / fp32 PSUM / fp16 out), v2:
  - Stationary = a 16x8 (h,w)-BLOCK of fm1 (K=64 channels, M=128 = 16*8
    pixels).  One stationary load then serves all 41*41 displacements for
    128 output pixels: the moving stream is fm2 rows [h0, h0+56) x padded
    cols [w0, w0+48), i.e. (16+40)*(8+40) = 2688 columns per load instead
    of the 8528 the per-row band formulation needs.  64 loads total.
  - Moving is streamed in 7 chunks of 8 rows x 48 cols = 384 fp32 columns,
    each into its own PSUM bank; PSUM[p = h_in*8+w_in, r_rel*48 + w_rel] =
    <fm1[:,h,w], fm2[:,h0+r_rel,w0+w_rel]>.
  - PSUM is evacuated (fp32->fp16) into an SBUF band tile S[128, 56, 48]
    split across DVE / ACT / Pool so no single engine bottlenecks.
  - Output DMA per load either writes the full band (TRIM=False) or uses a
    custom 3-dim addr64 access pattern whose outer stride advances 8
    partitions AND one 48-col band row at once (TRIM=True), storing only
    the 41 rows [h_in, h_in+41) each partition actually needs.
  - The w-direction shear (dx = w' - w_in) cannot be expressed in <=3 DMA
    dims; the host extracts the 41 diagonal columns with a zero-copy
    as_strided view during the fp32 upcast.
"""

import os
import sys

import numpy as np

for _p in ("/opt/trn_rl_repo",):
    if os.path.isdir(_p) and _p not in sys.path:
        sys.path.append(_p)

# ---- problem constants (hardcoded per contest rules) ----
B, C, H, W = 4, 64, 128, 128
MD = 20                  # max displacement
D = 2 * MD + 1           # 41 displacements per axis
PW = W + 2 * MD          # 168 padded width
HS = H // 2              # 64-row h-slab per core
RS = HS + 2 * MD         # 104 fm2 slab rows (with halo)
NCORES = 8

NH, NW = 16, 8           # fm1 pixel block per stationary load (NH*NW = 128)
NHB, NWB = HS // NH, W // NW      # 4 x 16 = 64 loads
BR, BC = NH + 2 * MD, NW + 2 * MD  # 56 x 48 band per load
CH = 8                   # moving rows per matmul chunk
NCH = BR // CH           # 7 chunks/load, 8*48 = 384 fp32 <= 512 (1 PSUM bank)

TRIM = True              # row-trimmed output DMA via custom addr64 AP

_CACHE = {}


def _patch_ldw_opt():
    """Flip walrus's --enable-ldw-opt to true for our compiles: all 7 matmuls
    of a load share one stationary, and without the opt each one pays a
    ~146ns LDWEIGHTS that serializes with streaming (~40% of PE time)."""
    from concourse import bass_utils

    if getattr(bass_utils, "_ldw_patched", False):
        return
    # NOTE: --enable-ldw-opt=true breaks walrus codegen (visitInstLdweights
    # error), so the redundant per-matmul LDWEIGHTS stays; it overlaps the
    # previous matmul's streaming via the PE shadow weight buffer.
    bass_utils._ldw_patched = True


def _build_program(io_dtype_name="float16", trim=TRIM):
    from concourse import bacc
    import concourse.mybir as mybir
    import concourse.tile as tile

    _patch_ldw_opt()

    dt_io = getattr(mybir.dt, io_dtype_name)

    nc = bacc.Bacc("TRN2", target_bir_lowering=False, debug=False)
    # fm1 host-bl blocked: [c, hb, wb, h_in*NW + w_in] so a stationary load is
    # one contiguous 128-wide slice (BIR: stationary AP = single free dim).
    fm1_d = nc.dram_tensor(
        "fm1s", [C, NHB, NWB, NH * NW], dt_io, kind="ExternalInput"
    ).ap()
    fm2_d = nc.dram_tensor("fm2s", [C, RS, PW], dt_io, kind="ExternalInput").ap()
    # trim: partitions 0-63 (h_in 0-7) only need band rows [0,48); partitions
    # 64-127 (h_in 8-15) only rows [8,56).  Two half-DMAs per load store
    # 48x48 per pixel instead of 56x48.
    TR = BR - CH         # 48 trimmed rows per half
    if trim:
        out_shape = [NHB * NWB, 2, 64, TR * BC]
    else:
        out_shape = [NHB * NWB, NH * NW, BR * BC]
    out_d = nc.dram_tensor("outs", out_shape, dt_io, kind="ExternalOutput").ap()

    with tile.TileContext(nc) as tc:
        with (
            tc.tile_pool(name="const", bufs=1) as cpool,
            tc.tile_pool(name="srow", bufs=3) as spool,
            tc.tile_pool(name="psum", bufs=1, space="PSUM") as ppool,
        ):
            fm1_sb = cpool.tile([C, NHB, NWB, NH * NW], dt_io)
            fm2_sb = cpool.tile([C, RS, PW], dt_io)
            nc.sync.dma_start(fm1_sb[:], fm1_d[:])
            nc.sync.dma_start(fm2_sb[:], fm2_d[:])

            # PSUM: 4 tile groups per load (1+2+2+2 chunk slots of 512 fp32,
            # each matmul in one bank).  Chunk 0 is only needed by partitions
            # 0-63 (its copy is half-width); pairs amortize the per-copy
            # overhead while keeping bank release fine-grained enough to
            # pipeline across loads.
            GSZ = (1, 2, 2, 2)
            for hb in range(NHB):
                for wb in range(NWB):
                    li = hb * NWB + wb
                    S = spool.tile([128, BR, BC], dt_io, tag="S")
                    j = 0
                    for gi, gn in enumerate(GSZ):
                        ps = ppool.tile(
                            [128, gn, 512], mybir.dt.float32,
                            name=f"ps{gi}", tag=f"ps{gi}",
                        )
                        for sl in range(gn):
                            nc.tensor.matmul(
                                ps[:, sl, 0 : CH * BC],
                                fm1_sb[:, hb, wb, :],
                                fm2_sb[:, NH * hb + CH * j
                                       : NH * hb + CH * (j + 1),
                                       NW * wb : NW * wb + BC],
                                start=True,
                                stop=True,
                            )
                            j += 1
                        if gi == 0:
                            # rows 0-8: only h_in 0-7 (partitions 0-63) use it
                            nc.vector.tensor_copy(
                                S[0:64, 0:CH, :].rearrange("p a b -> p (a b)"),
                                ps[0:64, 0, 0 : CH * BC],
                            )
                            continue
                        dst = S[:, CH * (j - gn) : CH * j, :].rearrange(
                            "p (g r) c -> p g (r c)", g=gn
                        )
                        copy = (
                            nc.vector.tensor_copy if gi == 2
                            else nc.scalar.copy
                        )
                        copy(dst, ps[:, :, 0 : CH * BC])
                    if trim:
                        nc.sync.dma_start(out_d[li, 0], S[0:64, 0:TR, :])
                        nc.gpsimd.dma_start(out_d[li, 1], S[64:128, CH:BR, :])
                    else:
                        nc.sync.dma_start(out_d[li], S[:])

    nc.compile()
    return nc


def _get_compiled(io_dtype_name="float16", trim=TRIM):
    key = ("prog", io_dtype_name, trim)
    if key not in _CACHE:
        _CACHE[key] = _build_program(io_dtype_name, trim)
    return _CACHE[key]


def shard_inputs(fm1, fm2, np_dtype=np.float16):
    """Full (4,64,128,128) inputs -> 8 per-core input dicts."""
    fm1 = np.asarray(fm1, dtype=np.float32)
    fm2 = np.asarray(fm2, dtype=np.float32)
    in_maps = []
    for k in range(NCORES):
        n, hbase = k // 2, (k % 2) * HS
        slab = fm1[n, :, hbase : hbase + HS].astype(np_dtype)  # (C, 64, 128)
        fm1s = np.ascontiguousarray(
            slab.reshape(C, NHB, NH, NWB, NW)
            .transpose(0, 1, 3, 2, 4)
            .reshape(C, NHB, NWB, NH * NW)
        )
        p = np.zeros((C, H + 2 * MD, PW), dtype=np_dtype)
        p[:, MD : MD + H, MD : MD + W] = fm2[n].astype(np_dtype)
        fm2s = np.ascontiguousarray(p[:, hbase : hbase + RS])  # (C, 104, 168)
        in_maps.append({"fm1s": fm1s, "fm2s": fm2s})
    return in_maps


def unshard_outputs(results, trim=TRIM):
    """8 per-core band outputs -> full (4,1,128,128,1681) fp32."""
    out = np.empty((B, 1, H, W, D * D), dtype=np.float32)
    for k in range(NCORES):
        n, hbase = k // 2, (k % 2) * HS
        g = np.asarray(results[k]["outs"])
        if trim:
            TR = BR - CH
            # [hb, wb, half, h_in', w_in, r - 8*half, w']
            a = g.reshape(NHB, NWB, 2, CH, NW, TR, BC)
            st = a.strides
            band = np.lib.stride_tricks.as_strided(
                a,
                shape=(NHB, NWB, 2, CH, NW, D, D),
                strides=(st[0], st[1], st[2], st[3] + st[5], st[4] + st[6],
                         st[5], st[6]),
            )
            out[n, 0, hbase : hbase + HS] = (
                band.transpose(0, 2, 3, 1, 4, 5, 6)
                .astype(np.float32)
                .reshape(HS, W, D * D)
            )
            continue
        else:
            a = g.reshape(NHB, NWB, NH, NW, BR, BC)
            st = a.strides
            band = np.lib.stride_tricks.as_strided(
                a,
                shape=(NHB, NWB, NH, NW, D, D),
                strides=(st[0], st[1], st[2] + st[4], st[3] + st[5],
                         st[4], st[5]),
            )
        out[n, 0, hbase : hbase + HS] = (
            band.transpose(0, 2, 1, 3, 4, 5)
            .astype(np.float32)
            .reshape(HS, W, D * D)
        )
    return out


def run_on_hw(in_maps, io_dtype_name="float16", trace=False, **kw):
    from concourse import bass_utils

    nc = _get_compiled(io_dtype_name)
    res = bass_utils.run_bass_kernel_spmd(
        nc, in_maps, list(range(NCORES)), trace=trace, **kw
    )
    return res


def kernel(feature_map_1, feature_map_2):
    in_maps = shard_inputs(feature_map_1, feature_map_2)
    res = run_on_hw(in_maps)
    return unshard_outputs(res.results)


if __name__ == "__main__":
    inputs = {
        "feature_map_1": np.random.randn(B, C, H, W).astype(np.float32),
        "feature_map_2": np.random.randn(B, C, H, W).astype(np.float32),
    }
    out = kernel(**inputs)
    print("kernel output", out.shape, out.dtype)


# revision 18
# speedup vs baseline: 1.3318x; 1.0531x over previous
"""Correlation-layer (cost volume) kernel for 8 Trainium2 NeuronCores.

Problem: out[n, 0, h, w, dy*41+dx] = sum_c fm1[n,c,h,w] * fm2p[n,c,h+dy,w+dx]
with fm2p = fm2 zero-padded by 20 on both spatial axes, dy,dx in [0,41).

Sharding: core k handles batch n = k//2 and h-slab [64*(k%2), 64*(k%2)+64).
No cross-core communication: each core's fm2 slab (with a 20-row halo) is
prepared on the host.

Device algorithm (per core, fp16 in / fp32 PSUM / fp16 out), v2:
  - Stationary = a 16x8 (h,w)-BLOCK of fm1 (K=64 channels, M=128 = 16*8
    pixels).  One stationary load then serves all 41*41 displacements for
    128 output pixels: the moving stream is fm2 rows [h0, h0+56) x padded
    cols [w0, w0+48), i.e. (16+40)*(8+40) = 2688 columns per load instead
    of the 8528 the per-row band formulation needs.  64 loads total.
  - Moving is streamed in 7 chunks of 8 rows x 48 cols = 384 fp32 columns,
    each into its own PSUM bank; PSUM[p = h_in*8+w_in, r_rel*48 + w_rel] =
    <fm1[:,h,w], fm2[:,h0+r_rel,w0+w_rel]>.
  - PSUM is evacuated (fp32->fp16) into an SBUF band tile S[128, 56, 48]
    split across DVE / ACT / Pool so no single engine bottlenecks.
  - Output DMA per load either writes the full band (TRIM=False) or uses a
    custom 3-dim addr64 access pattern whose outer stride advances 8
    partitions AND one 48-col band row at once (TRIM=True), storing only
    the 41 rows [h_in, h_in+41) each partition actually needs.
  - The w-direction shear (dx = w' - w_in) cannot be expressed in <=3 DMA
    dims; the host extracts the 41 diagonal columns with a zero-copy
    as_strided view during the fp32 upcast.
"""

import os
import sys

import numpy as np

for _p in ("/opt/trn_rl_repo",):
    if os.path.isdir(_p) and _p not in sys.path:
        sys.path.append(_p)

# ---- problem constants (hardcoded per contest rules) ----
B, C, H, W = 4, 64, 128, 128
MD = 20                  # max displacement
D = 2 * MD + 1           # 41 displacements per axis
PW = W + 2 * MD          # 168 padded width
HS = H // 2              # 64-row h-slab per core
RS = HS + 2 * MD         # 104 fm2 slab rows (with halo)
NCORES = 8

NH, NW = 16, 8           # fm1 pixel block per stationary load (NH*NW = 128)
NHB, NWB = HS // NH, W // NW      # 4 x 16 = 64 loads
BR, BC = NH + 2 * MD, NW + 2 * MD  # 56 x 48 band per load
CH = 8                   # moving rows per matmul chunk
NCH = BR // CH           # 7 chunks/load, 8*48 = 384 fp32 <= 512 (1 PSUM bank)

TRIM = True              # row-trimmed output DMA via custom addr64 AP

_CACHE = {}


def _patch_ldw_opt():
    """Flip walrus's --enable-ldw-opt to true for our compiles: all 7 matmuls
    of a load share one stationary, and without the opt each one pays a
    ~146ns LDWEIGHTS that serializes with streaming (~40% of PE time)."""
    from concourse import bass_utils

    if getattr(bass_utils, "_ldw_patched", False):
        return
    # NOTE: --enable-ldw-opt=true breaks walrus codegen (visitInstLdweights
    # error), so the redundant per-matmul LDWEIGHTS stays; it overlaps the
    # previous matmul's streaming via the PE shadow weight buffer.
    bass_utils._ldw_patched = True


def _build_program(io_dtype_name="float16", trim=TRIM):
    from concourse import bacc
    import concourse.mybir as mybir
    import concourse.tile as tile

    _patch_ldw_opt()

    dt_io = getattr(mybir.dt, io_dtype_name)

    nc = bacc.Bacc("TRN2", target_bir_lowering=False, debug=False)
    # fm1 host-bl blocked: [c, hb, wb, h_in*NW + w_in] so a stationary load is
    # one contiguous 128-wide slice (BIR: stationary AP = single free dim).
    fm1_d = nc.dram_tensor(
        "fm1s", [C, NHB, NWB, NH * NW], dt_io, kind="ExternalInput"
    ).ap()
    fm2_d = nc.dram_tensor("fm2s", [C, RS, PW], dt_io, kind="ExternalInput").ap()
    # Output DMA batches 2 consecutive loads into one transfer so each
    # partition contributes a 10.75KB contiguous run -- small (<5KB) runs
    # measured only ~257 GB/s vs ~340 GB/s for >10KB runs.
    TR = BR - CH
    out_shape = [NHB * NWB // 2, NH * NW, 2, BR * BC]
    out_d = nc.dram_tensor("outs", out_shape, dt_io, kind="ExternalOutput").ap()

    with tile.TileContext(nc) as tc:
        with (
            tc.tile_pool(name="const", bufs=1) as cpool,
            tc.tile_pool(name="srow", bufs=3) as spool,
            tc.tile_pool(name="psum", bufs=1, space="PSUM") as ppool,
        ):
            fm1_sb = cpool.tile([C, NHB, NWB, NH * NW], dt_io)
            fm2_sb = cpool.tile([C, RS, PW], dt_io)
            nc.sync.dma_start(fm1_sb[:], fm1_d[:])
            nc.sync.dma_start(fm2_sb[:], fm2_d[:])

            # 7 fine-grained chunks per load: one PSUM bank each, copied out
            # individually so banks release early and the PE never waits
            # long.  Chunk 0 is only needed by partitions 0-63 and chunk 6
            # only by 64-127, so their copies are half-width (on ACT).
            for hb in range(NHB):
                for wb in range(NWB):
                    li = hb * NWB + wb
                    if li % 2 == 0:
                        S2 = spool.tile([128, 2, BR, BC], dt_io, tag="S")
                    S = S2[:, li % 2]
                    for j in range(NCH):
                        ps = ppool.tile(
                            [128, CH, BC], mybir.dt.float32,
                            name=f"ps{j}", tag=f"ps{j}",
                        )
                        nc.tensor.matmul(
                            ps[:],
                            fm1_sb[:, hb, wb, :],
                            fm2_sb[:, NH * hb + CH * j : NH * hb + CH * (j + 1),
                                   NW * wb : NW * wb + BC],
                            start=True,
                            stop=True,
                        )
                        if j == 0:
                            nc.scalar.copy(S[0:64, 0:CH, :], ps[0:64])
                        elif j == NCH - 1:
                            nc.scalar.copy(
                                S[64:128, CH * j : CH * (j + 1), :], ps[64:128]
                            )
                        else:
                            copy = (
                                nc.scalar.copy if j % 2 == 0
                                else nc.vector.tensor_copy
                            )
                            copy(S[:, CH * j : CH * (j + 1), :], ps[:])
                    if li % 2 == 1:
                        nc.sync.dma_start(out_d[li // 2], S2[:])

    nc.compile()
    return nc


def _get_compiled(io_dtype_name="float16", trim=TRIM):
    key = ("prog", io_dtype_name, trim)
    if key not in _CACHE:
        _CACHE[key] = _build_program(io_dtype_name, trim)
    return _CACHE[key]


def shard_inputs(fm1, fm2, np_dtype=np.float16):
    """Full (4,64,128,128) inputs -> 8 per-core input dicts."""
    fm1 = np.asarray(fm1, dtype=np.float32)
    fm2 = np.asarray(fm2, dtype=np.float32)
    in_maps = []
    for k in range(NCORES):
        n, hbase = k // 2, (k % 2) * HS
        slab = fm1[n, :, hbase : hbase + HS].astype(np_dtype)  # (C, 64, 128)
        fm1s = np.ascontiguousarray(
            slab.reshape(C, NHB, NH, NWB, NW)
            .transpose(0, 1, 3, 2, 4)
            .reshape(C, NHB, NWB, NH * NW)
        )
        p = np.zeros((C, H + 2 * MD, PW), dtype=np_dtype)
        p[:, MD : MD + H, MD : MD + W] = fm2[n].astype(np_dtype)
        fm2s = np.ascontiguousarray(p[:, hbase : hbase + RS])  # (C, 104, 168)
        in_maps.append({"fm1s": fm1s, "fm2s": fm2s})
    return in_maps


def unshard_outputs(results, trim=TRIM):
    """8 per-core band outputs -> full (4,1,128,128,1681) fp32."""
    out = np.empty((B, 1, H, W, D * D), dtype=np.float32)
    for k in range(NCORES):
        n, hbase = k // 2, (k % 2) * HS
        g = np.asarray(results[k]["outs"])
        # [hb, wb-pair, h_in, w_in, sub, r, w']
        a = g.reshape(NHB, NWB // 2, NH, NW, 2, BR, BC)
        st = a.strides
        band = np.lib.stride_tricks.as_strided(
            a,
            shape=(NHB, NWB // 2, NH, NW, 2, D, D),
            strides=(st[0], st[1], st[2] + st[5], st[3] + st[6],
                     st[4], st[5], st[6]),
        )
        out[n, 0, hbase : hbase + HS] = (
            band.transpose(0, 2, 1, 4, 3, 5, 6)
            .astype(np.float32)
            .reshape(HS, W, D * D)
        )
    return out


def run_on_hw(in_maps, io_dtype_name="float16", trace=False, **kw):
    from concourse import bass_utils

    nc = _get_compiled(io_dtype_name)
    res = bass_utils.run_bass_kernel_spmd(
        nc, in_maps, list(range(NCORES)), trace=trace, **kw
    )
    return res


def kernel(feature_map_1, feature_map_2):
    in_maps = shard_inputs(feature_map_1, feature_map_2)
    res = run_on_hw(in_maps)
    return unshard_outputs(res.results)


if __name__ == "__main__":
    inputs = {
        "feature_map_1": np.random.randn(B, C, H, W).astype(np.float32),
        "feature_map_2": np.random.randn(B, C, H, W).astype(np.float32),
    }
    out = kernel(**inputs)
    print("kernel output", out.shape, out.dtype)
